# revision 1
# baseline (speedup 1.0000x reference)
"""Multi-head attention Trainium2 Bass kernel.

Problem: B=8, N=2048, C=768, H=12 heads, D=64 head dim.
  qkv = x @ w_qkv.T          -> [B, N, 3C]
  per head: softmax(q k^T / sqrt(D)) @ v
  y = attn_out @ w_proj.T + b_proj

Sharding: data parallel over batch — one batch element per NeuronCore (8 cores).

Per-core layout strategy (everything "transposed", feature-major):
  xT/w_qkvT/w_projT arrive pre-transposed from the host (free in numpy)
  qkvT [F, N] = W_qkv^T-stationary matmuls over xT   (F = 3C = 2304)
  S^T  [nk, nq] per head = kT-tile-stationary vs qT moving -> the softmax
       denominator comes from a ones-column appended to V in the A@V matmul
       (row 64 of the AV psum accumulates sum(exp(s))).
  exp via ScalarE (scale=1/8 folded in, no max subtraction: |scores| <~ 2.5)
  aT   [C, N] normalized attention output, fed as lhsT to the proj matmul.

Fully fused: each head pair's q/k/v is produced on-chip (w_qkvT f-tile
slices and xT chunks streamed from DRAM, no qkvT scratch round-trip); those
matmuls are dependency-free PE filler under the ScalarE exp chain, leaving
the kernel PE-bound at ~99% duty. The two heads of a pair occupy SBUF
partitions 0-63 / 64-127, and their S^T matmuls are interleaved per nk-tile
so adjacent instructions hit disjoint PE row groups (hardware overlaps the
two K=64 streams). Softmax normalization uses gpsimd partition_broadcast;
projection shares the attention scope and borrows the idle qkv psum pool.

All matmuls run in float32r (~1 cycle/row at free dim >= 256, rel err ~2e-4).
"""

import numpy as np

import concourse.bass as bass
import concourse.mybir as mybir
import concourse.tile as tile
from concourse import bacc
from concourse.bass_utils import run_bass_kernel_spmd
from concourse.masks import make_identity

B, N, C, H = 8, 2048, 768, 12
D = C // H            # 64
F = 3 * C             # 2304
NT = N // 128         # 16 seq tiles
CT = C // 128         # 6 channel tiles
FT = F // 128         # 18 qkv-feature tiles
NQ = 512              # query-chunk width (1 psum bank of fp32)
NCH = N // NQ         # 4 chunks
SCALE = float(D) ** -0.5

FP32 = mybir.dt.float32
FP32R = mybir.dt.float32r
EXP = mybir.ActivationFunctionType.Exp

_CACHED_NC = None


def _bc_ap(dram_ap, parts):
    """Partition-broadcast a 1-D DRAM AP to [parts, len] via stride-0."""
    return bass.AP(
        tensor=dram_ap.tensor,
        offset=dram_ap.offset,
        ap=[[0, parts]] + [list(p) for p in dram_ap.ap],
    )


def build():
    # xT/w_qkvT/w_projT arrive pre-transposed (feature-major) from the host:
    # the layout change is free in numpy and removes every input transpose
    # (PE + ScalarE evict) from the device timeline.
    nc = bacc.Bacc()
    x = nc.dram_tensor("xT", [C, N], FP32, kind="ExternalInput")
    w_qkv = nc.dram_tensor("w_qkvT", [C, F], FP32, kind="ExternalInput")
    w_proj = nc.dram_tensor("w_projT", [C, C], FP32, kind="ExternalInput")
    b_proj = nc.dram_tensor("b_proj", [C], FP32, kind="ExternalInput")
    y = nc.dram_tensor("y", [N, C], FP32, kind="ExternalOutput")
    aT_d = nc.dram_tensor("aT_scratch", [C, N], FP32R)

    xr = x[:, :].bitcast(FP32R)
    wqr = w_qkv[:, :].bitcast(FP32R)
    wpr = w_proj[:, :].bitcast(FP32R)

    lp = nc.allow_low_precision("float32r psum accumulation is fp32-width")
    lp.__enter__()
    with tile.TileContext(nc) as tc:
        const_cm = tc.tile_pool(name="const", bufs=1)
        const = const_cm.__enter__()
        ident_f = const.tile([128, 128], FP32)
        make_identity(nc, ident_f)
        ident = const.tile([128, 128], FP32R)
        nc.vector.tensor_copy(ident, ident_f)
        ones_row_f = const.tile([1, D], FP32)
        nc.vector.memset(ones_row_f, 1.0)
        ones_row = const.tile([1, D], FP32R)
        nc.vector.tensor_copy(ones_row, ones_row_f)
        ones_col = const.tile([128, NT, 1], FP32)
        nc.vector.memset(ones_col, 1.0)
        xr3 = xr.rearrange("(ko p) n -> p ko n", p=128)
        wqr3 = wqr.rearrange("(ko p) f -> p ko f", p=128)

        # ---------------- phase 2: attention, head pairs --------------------
        with tc.tile_pool(name="hpool", bufs=2) as hpool, \
             tc.tile_pool(name="spool", bufs=1) as spool, \
             tc.tile_pool(name="small", bufs=2) as small, \
             tc.tile_pool(name="psum_s", bufs=2, space="PSUM") as psum_s, \
             tc.tile_pool(name="psum_av", bufs=2, space="PSUM") as psum_av, \
             tc.tile_pool(name="psum_qkv", bufs=2, space="PSUM") as psum_qkv:

            for hp in range(H // 2):
                # produce this pair's q/k/v on-chip: stream the three w_qkvT
                # f-tiles {hp, 6+hp, 12+hp} and x chunks from DRAM; the qkv
                # matmuls are dependency-free PE filler under the exp chain.
                wqs = []
                for idx, m in enumerate((hp, CT + hp, 2 * CT + hp)):
                    w = hpool.tile(
                        [128, CT, 128], FP32R, tag=f"wq{idx}", name=f"wq{idx}",
                        bufs=1,
                    )
                    nc.sync.dma_start(
                        out=w, in_=wqr3[:, :, m * 128:(m + 1) * 128]
                    )
                    wqs.append(w)
                qTt = hpool.tile([128, N], FP32R, tag="qT")
                kTt = hpool.tile([128, N], FP32R, tag="kT")
                vTt = hpool.tile([128, N], FP32R, tag="vT")
                qkvts = (qTt, kTt, vTt)
                for j in range(NCH):
                    xc = hpool.tile([128, CT, NQ], FP32R, tag="xc", name="xc")
                    nc.sync.dma_start(
                        out=xc, in_=xr3[:, :, j * NQ:(j + 1) * NQ]
                    )
                    for idx in range(3):
                        ps = psum_qkv.tile([128, NQ], FP32, tag="qkvps", name="qkvps")
                        for k in range(CT):
                            nc.tensor.matmul(
                                ps,
                                wqs[idx][:, k, :],
                                xc[:, k, :],
                                start=(k == 0),
                                stop=(k == CT - 1),
                            )
                        nc.vector.tensor_copy(
                            qkvts[idx][:, j * NQ:(j + 1) * NQ], ps
                        )
                vaugs = []
                for a in range(2):
                    vaug = hpool.tile([128, NT, D + 1], FP32R, tag=f"vaug{a}")
                    nc.vector.tensor_copy(vaug[:, :, D:D + 1], ones_col)
                    vaugs.append(vaug)
                # A/B transposes interleaved per tile: adjacent PE
                # instructions hit disjoint row groups (0-63 / 64-127)
                for t0 in range(0, NT, 8):
                    pts = [
                        psum_av.tile(
                            [128, 8, D], FP32R, tag="av", name=f"pt{a}"
                        )
                        for a in range(2)
                    ]
                    for g in range(8):
                        t = t0 + g
                        for a in range(2):
                            lo = a * D
                            nc.tensor.transpose(
                                pts[a][:, g, :],
                                vTt[lo:lo + D, t * 128:(t + 1) * 128],
                                ident[lo:lo + D, lo:lo + D],
                            )
                    for a in range(2):
                        nc.vector.tensor_copy(
                            vaugs[a][:, t0:t0 + 8, 0:D], pts[a]
                        )

                # nk-tile group sizes: 3-bank psum tiles double-buffered so
                # ScalarE exp(g) overlaps the S^T matmuls of g+1.
                GROUPS = (2, 2, 2, 2, 2, 2, 2, 2)
                for j in range(NCH):
                    expSs = [
                        spool.tile(
                            [128, NT, NQ], FP32R,
                            tag=f"expS{a}", name=f"expS{a}",
                        )
                        for a in range(2)
                    ]
                    t = 0
                    for gsz in GROUPS:
                        # the two heads' matmuls are interleaved per nk-tile:
                        # adjacent MMs target disjoint PE row groups
                        # (partitions 0-63 / 64-127) and overlap in the array
                        sps_ab = [
                            psum_s.tile(
                                [128, 2, NQ], FP32, tag=f"sps{a}",
                                name=f"sps{a}", bufs=1,
                            )
                            for a in range(2)
                        ]
                        for u in range(gsz):
                            for a in range(2):
                                lo = a * D
                                nc.tensor.matmul(
                                    sps_ab[a][:, u, :],
                                    kTt[lo:lo + D, (t + u) * 128:(t + u + 1) * 128],
                                    qTt[lo:lo + D, j * NQ:(j + 1) * NQ],
                                    start=True,
                                    stop=True,
                                )
                        for a in range(2):
                            nc.scalar.activation(
                                out=expSs[a][:, t:t + gsz, :],
                                in_=sps_ab[a][:, 0:gsz, :],
                                func=EXP,
                                scale=SCALE,
                            )
                        t += gsz
                    for a in range(2):
                        h = 2 * hp + a
                        av = psum_av.tile([D + 1, NQ], FP32, tag="av")
                        for t in range(NT):
                            nc.tensor.matmul(
                                av,
                                vaugs[a][:, t, :],
                                expSs[a][:, t, :],
                                start=(t == 0),
                                stop=(t == NT - 1),
                            )
                        recip = small.tile([1, NQ], FP32, tag="recip")
                        nc.vector.reciprocal(recip, av[D:D + 1, :])
                        bc_sb = small.tile([D, NQ], FP32, tag="bc_sb")
                        nc.gpsimd.partition_broadcast(bc_sb, recip)
                        aTt = small.tile([D, NQ], FP32R, tag="aT_sb")
                        nc.vector.tensor_mul(aTt, av[0:D, :], bc_sb)
                        nc.sync.dma_start(
                            out=aT_d[h * D:(h + 1) * D, j * NQ:(j + 1) * NQ],
                            in_=aTt,
                        )

            # ---------- phase 3: output projection, inside the same scope.
            # proj psums borrow the qkv pool (idle once the last pair's
            # q/k/v are built), so proj matmuls fill the attention tail.
            bias_bc = small.tile([128, C], FP32, tag="bias", bufs=1)
            nc.gpsimd.dma_start(out=bias_bc, in_=_bc_ap(b_proj[:], 128))
            w_projT = small.tile([128, CT, C], FP32R, tag="wproj", bufs=1)
            nc.sync.dma_start(
                out=w_projT, in_=wpr.rearrange("(ko p) o -> p ko o", p=128)
            )
            NO = 384
            for i in range(NT):
                a_sb = small.tile([128, CT, 128], FP32R, tag="a_sb", bufs=2)
                nc.sync.dma_start(
                    out=a_sb,
                    in_=aT_d[:, i * 128:(i + 1) * 128].rearrange(
                        "(ko p) n -> p ko n", p=128
                    ),
                )
                for half in range(2):
                    ps = psum_qkv.tile([128, NO], FP32, tag="qkvps")
                    for k in range(CT):
                        nc.tensor.matmul(
                            ps,
                            a_sb[:, k, :],
                            w_projT[:, k, half * NO:(half + 1) * NO],
                            start=(k == 0),
                            stop=(k == CT - 1),
                        )
                    y_sb = small.tile([128, NO], FP32, tag="y_sb", bufs=2)
                    nc.vector.tensor_add(
                        y_sb, ps, bias_bc[:, half * NO:(half + 1) * NO]
                    )
                    nc.sync.dma_start(
                        out=y[i * 128:(i + 1) * 128, half * NO:(half + 1) * NO],
                        in_=y_sb,
                    )
        const_cm.__exit__(None, None, None)
    lp.__exit__(None, None, None)

    nc.finalize()
    return nc


def get_nc():
    global _CACHED_NC
    if _CACHED_NC is None:
        _CACHED_NC = build()
    return _CACHED_NC


LAST_RESULT = None


def kernel(x, w_qkv, w_proj, b_proj, **run_kwargs):
    x = np.ascontiguousarray(np.asarray(x, dtype=np.float32))
    w_qkv = np.ascontiguousarray(np.asarray(w_qkv, dtype=np.float32))
    w_proj = np.ascontiguousarray(np.asarray(w_proj, dtype=np.float32))
    b_proj = np.ascontiguousarray(np.asarray(b_proj, dtype=np.float32))
    assert x.shape == (B, N, C)

    nc = get_nc()
    w_qkvT = np.ascontiguousarray(w_qkv.T)
    w_projT = np.ascontiguousarray(w_proj.T)
    in_maps = [
        {
            "xT": np.ascontiguousarray(x[i].T),
            "w_qkvT": w_qkvT,
            "w_projT": w_projT,
            "b_proj": b_proj,
        }
        for i in range(B)
    ]
    res = run_bass_kernel_spmd(nc, in_maps, list(range(B)), **run_kwargs)
    global LAST_RESULT
    LAST_RESULT = res
    out = np.stack([res.results[i]["y"] for i in range(B)], axis=0)
    return out


if __name__ == "__main__":
    rng = np.random.default_rng(0)
    x = rng.standard_normal((B, N, C), dtype=np.float32)
    w_qkv = (rng.standard_normal((F, C)) * 0.02).astype(np.float32)
    w_proj = (rng.standard_normal((C, C)) * 0.02).astype(np.float32)
    b_proj = (rng.standard_normal((C,)) * 0.02).astype(np.float32)
    out = kernel(x=x, w_qkv=w_qkv, w_proj=w_proj, b_proj=b_proj)
    print("out", out.shape, out.dtype, float(np.abs(out).max()))



# revision 21
# speedup vs baseline: 1.1994x; 1.1994x over previous
"""Multi-head attention Trainium2 Bass kernel (fp8-DoubleRow version).

Problem: B=8, N=2048, C=768, H=12 heads, D=64 head dim.
  qkv = x @ w_qkv.T          -> [B, N, 3C]
  per head: softmax(q k^T / sqrt(D)) @ v
  y = attn_out @ w_proj.T + b_proj

Sharding: data parallel over batch - one batch element per NeuronCore.

Mixed-precision strategy (cost model: matmul = out_free x pe_cycle x cpr,
fp8 DoubleRow cpr=0.5 vs fp32r 1.0):
  - QKV: fp32r (exact q/k/v; fp8 here costs too much accuracy).
  - S = k^T q: fp8 DoubleRow. lhsT = k8 duplicated via a stride-0 block dim,
    rhs blocks = (q_hi, q_lo) -> S = k8^T (q_hi + q_lo): q at ~14-bit
    precision, k at fp8, half the fp32r PE cost.
  - exp: 12/16 kv-tiles on ScalarE (activation Exp -> fp8 out), 4/16 via a
    Schraudolph fast-exp: DVE affine to int16 (bf16 exponent bits), gpsimd
    converts bf16 -> fp8e4m3.
  - AV: fp8 DoubleRow pairing kv-tiles. lhsT M-dim = [v_hi d0-63 | ones |
    v_lo d1-63]: v at ~14-bit, denominator row free (psum row 64).
  - normalize: tmp = copy(av[64:128]) (denominator lands in tmp[0]),
    recip + partition_broadcast, t = av[0:64] + tmp (row d0 absorbs +denom
    -> exactly +1 after the reciprocal multiply; corrected by subtracting
    sum_h w_proj[:, 64h] from b_proj on the host), aT = t * bc_r.
  - proj: fp32r, unchanged from the fp32r kernel.
"""

import numpy as np

import concourse.bass as bass
import concourse.mybir as mybir
import concourse.tile as tile
from concourse import bacc
from concourse.bass_utils import run_bass_kernel_spmd
from concourse.masks import make_identity

B, N, C, H = 8, 2048, 768, 12
D = C // H            # 64
F = 3 * C             # 2304
NT = N // 128         # 16 seq tiles
CT = C // 128         # 6 channel tiles
NQ = 512              # query-chunk width (1 psum bank of fp32)
NCH = N // NQ         # 4 chunks
SCALE = float(D) ** -0.5

FP32 = mybir.dt.float32
FP32R = mybir.dt.float32r
FP8 = mybir.dt.float8e4
I16 = mybir.dt.int16
BF16 = mybir.dt.bfloat16
EXP = mybir.ActivationFunctionType.Exp
DR = mybir.MatmulPerfMode.DoubleRow
ADD = mybir.AluOpType.add
SUB = mybir.AluOpType.subtract
MULT = mybir.AluOpType.mult

# Schraudolph constants: bf16 bits = 128*log2(exp(s_eff)) + 127*128,
# s_eff = S_psum * SCALE -> bits = S * (128*SCALE/ln2) + 16256.
SCHR_A = 128.0 * SCALE / float(np.log(2.0))
SCHR_B = 16256.0
N_SCHR_GROUPS = 2      # of 8 groups of 2 kv-tiles: last N on DVE+Pool

_CACHED_NC = None


def _dup2(ap):
    """Insert a stride-0 [0,2] block dim after the partition dim."""
    return bass.AP(
        tensor=ap.tensor,
        offset=ap.offset,
        ap=[list(ap.ap[0]), [0, 2]] + [list(d) for d in ap.ap[1:]],
    )


def _bc_ap(dram_ap, parts):
    """Partition-broadcast a 1-D DRAM AP to [parts, len] via stride-0."""
    return bass.AP(
        tensor=dram_ap.tensor,
        offset=dram_ap.offset,
        ap=[[0, parts]] + [list(p) for p in dram_ap.ap],
    )


def build():
    nc = bacc.Bacc()
    x = nc.dram_tensor("xT", [C, N], FP32, kind="ExternalInput")
    w_qkv = nc.dram_tensor("w_qkvT", [C, F], FP32, kind="ExternalInput")
    w_proj = nc.dram_tensor("w_projT", [C, C], FP32, kind="ExternalInput")
    b_proj = nc.dram_tensor("b_proj", [C], FP32, kind="ExternalInput")
    y = nc.dram_tensor("y", [N, C], FP32, kind="ExternalOutput")
    aT_d = nc.dram_tensor("aT_scratch", [C, N], FP32R)

    xr = x[:, :].bitcast(FP32R)
    wqr = w_qkv[:, :].bitcast(FP32R)
    wpr = w_proj[:, :].bitcast(FP32R)

    lp = nc.allow_low_precision("fp8 attention with hi/lo compensation")
    lp.__enter__()
    with tile.TileContext(nc) as tc:
        const_cm = tc.tile_pool(name="const", bufs=1)
        const = const_cm.__enter__()
        ident_f = const.tile([128, 128], FP32)
        make_identity(nc, ident_f)
        ident = const.tile([128, 128], FP32R)
        nc.vector.tensor_copy(ident, ident_f)
        xr3 = xr.rearrange("(ko p) n -> p ko n", p=128)
        wqr3 = wqr.rearrange("(ko p) f -> p ko f", p=128)

        with tc.tile_pool(name="hpool", bufs=2) as hpool, \
             tc.tile_pool(name="spool", bufs=1) as spool, \
             tc.tile_pool(name="small", bufs=2) as small, \
             tc.tile_pool(name="psum_s", bufs=2, space="PSUM") as psum_s, \
             tc.tile_pool(name="psum_av", bufs=2, space="PSUM") as psum_av, \
             tc.tile_pool(name="psum_qkv", bufs=1, space="PSUM") as psum_qkv:

            def qkv_psum(alt):
                return psum_qkv.tile([128, NQ], FP32, tag="qkvps",
                                     name="qkvps")

            def emit_qkv_vaug(hp):
                # ---- produce this pair's q/k/v on-chip (fp32r matmuls)
                wq = hpool.tile(
                    [128, CT, 3, 128], FP32R, tag="wq", name="wq", bufs=1,
                )
                for idx, m in enumerate((hp, CT + hp, 2 * CT + hp)):
                    nc.sync.dma_start(
                        out=wq[:, :, idx, :],
                        in_=wqr3[:, :, m * 128:(m + 1) * 128],
                    )
                qTt = hpool.tile([128, 2, N], FP8, tag="qT")     # hi/lo blocks
                kTt = hpool.tile([128, N], FP8, tag="kT")
                vTt = hpool.tile([128, N], FP32R, tag="vT")
                for j in range(NCH):
                    xc = hpool.tile([128, CT, NQ], FP32R, tag="xc", name="xc")
                    nc.sync.dma_start(
                        out=xc, in_=xr3[:, :, j * NQ:(j + 1) * NQ]
                    )
                    js = slice(j * NQ, (j + 1) * NQ)
                    for idx in range(3):
                        ps = qkv_psum(idx)
                        for k in range(CT):
                            nc.tensor.matmul(
                                ps,
                                wq[:, k, idx, :],
                                xc[:, k, :],
                                start=(k == 0),
                                stop=(k == CT - 1),
                            )
                        if idx == 0:      # q -> hi + lo fp8
                            nc.vector.tensor_copy(qTt[:, 0, js], ps)
                            nc.vector.tensor_tensor(
                                qTt[:, 1, js], ps, qTt[:, 0, js], SUB
                            )
                        elif idx == 1:    # k -> fp8
                            nc.vector.tensor_copy(kTt[:, js], ps)
                        else:             # v -> fp32r (transposed later)
                            nc.vector.tensor_copy(vTt[:, js], ps)

                # ---- v transposes; vaug = [v_hi d0-63 | ones | v_lo d1-63]
                vaugs = []
                for a in range(2):
                    vaug = hpool.tile([128, NT, 128], FP8, tag=f"vaug{a}")
                    nc.vector.memset(vaug[:, :, D:D + 1], 1.0)
                    vaugs.append(vaug)
                for t0 in range(0, NT, 8):
                    pts = [
                        psum_av.tile(
                            [128, 8, D], FP32R, tag="av", name=f"pt{a}"
                        )
                        for a in range(2)
                    ]
                    for g in range(8):
                        t = t0 + g
                        for a in range(2):
                            lo = a * D
                            nc.tensor.transpose(
                                pts[a][:, g, :],
                                vTt[lo:lo + D, t * 128:(t + 1) * 128],
                                ident[lo:lo + D, lo:lo + D],
                            )
                    for a in range(2):
                        nc.vector.tensor_copy(
                            vaugs[a][:, t0:t0 + 8, 0:D], pts[a]
                        )
                        nc.vector.tensor_tensor(
                            vaugs[a][:, t0:t0 + 8, D + 1:128],
                            pts[a][:, :, 1:D],
                            vaugs[a][:, t0:t0 + 8, 1:D],
                            SUB,
                        )
                return qTt, kTt, vaugs

            def emit_attention(hp, qTt, kTt, vaugs, post_j=None):
                # ---- attention per q-chunk
                aTts = [
                    hpool.tile([D, N], FP32, tag=f"aTt{a}", name=f"aTt{a}")
                    for a in range(2)
                ]
                for j in range(NCH):
                    js = slice(j * NQ, (j + 1) * NQ)
                    expSs = [
                        spool.tile(
                            [128, NT, NQ], FP8,
                            tag=f"expS{a}", name=f"expS{a}", bufs=2,
                        )
                        for a in range(2)
                    ]
                    def s_mm(out_ap, a, t):
                        lo = a * D
                        kap = kTt[lo:lo + D, t * 128:(t + 1) * 128]
                        nc.tensor.matmul(
                            out_ap,
                            _dup2(kap),
                            qTt[lo:lo + D, :, js],
                            start=True,
                            stop=True,
                            perf_mode=DR,
                        )

                    def emit_act_group(g):
                        # tiles (2g, 2g+1) -> ScalarE exp, own sps0/1 ring
                        t = 2 * g
                        sps_ab = [
                            psum_s.tile(
                                [128, 2, NQ], FP32, tag=f"sps{a}",
                                name=f"sps{a}", bufs=1,
                            )
                            for a in range(2)
                        ]
                        for u in range(2):
                            for a in range(2):
                                s_mm(sps_ab[a][:, u, :], a, t + u)
                        for a in range(2):
                            nc.scalar.activation(
                                out=expSs[a][:, t:t + 2, :],
                                in_=sps_ab[a][:, :, :],
                                func=EXP,
                                scale=SCALE,
                            )

                    def emit_schr_tile(t):
                        # single kv-tile -> DVE schraudolph + Pool convert,
                        # separate 1-bank psum ring (never gates ScalarE)
                        for a in range(2):
                            spsx = psum_s.tile(
                                [128, NQ], FP32, tag="spsX",
                                name="spsX", bufs=1,
                            )
                            s_mm(spsx, a, t)
                            i16 = small.tile(
                                [128, NQ], I16, tag=f"i16_{a}",
                                name=f"i16_{a}",
                            )
                            nc.vector.tensor_scalar(
                                i16, spsx, SCHR_A, SCHR_B, MULT, ADD,
                            )
                            nc.gpsimd.tensor_copy(
                                expSs[a][:, t:t + 1, :],
                                i16.bitcast(BF16),
                            )

                    # act-ring tiles 0-11, schr tiles 12-15, interleaved so
                    # DVE picks up schraudolph work early
                    emit_schr_tile(12)
                    emit_act_group(0)
                    emit_schr_tile(13)
                    emit_act_group(1)
                    emit_schr_tile(14)
                    emit_act_group(2)
                    emit_schr_tile(15)
                    for g in range(3, 6):
                        emit_act_group(g)
                    for a in range(2):
                        av = psum_av.tile([128, NQ], FP32, tag="av")
                        for u in range(NT // 2):
                            nc.tensor.matmul(
                                av,
                                vaugs[a][:, 2 * u:2 * u + 2, :],
                                expSs[a][:, 2 * u:2 * u + 2, :],
                                start=(u == 0),
                                stop=(u == NT // 2 - 1),
                                perf_mode=DR,
                            )
                        # tmp = [denom | v_lo part]: row0 = denominators
                        tmp = small.tile([D, NQ], FP32, tag="tmp")
                        nc.vector.tensor_copy(tmp, av[D:128, :])
                        recip = small.tile([1, NQ], FP32, tag="recip")
                        nc.vector.reciprocal(recip, tmp[0:1, :])
                        bc_sb = small.tile([D, NQ], FP32, tag="bc_sb")
                        nc.gpsimd.partition_broadcast(bc_sb, recip)
                        t64 = small.tile([D, NQ], FP32, tag="t64")
                        nc.vector.tensor_tensor(t64, av[0:D, :], tmp, ADD)
                        nc.gpsimd.tensor_tensor(
                            aTts[a][:, js], t64, bc_sb, MULT
                        )
                    # half-pair aT stores let the proj overlap the tail of
                    # the last pair's attention
                    if j % 2 == 1:
                        js2 = slice((j - 1) * NQ, (j + 1) * NQ)
                        for a in range(2):
                            h = 2 * hp + a
                            nc.sync.dma_start(
                                out=aT_d[h * D:(h + 1) * D, js2].bitcast(FP32),
                                in_=aTts[a][:, js2],
                            )
                    if post_j is not None:
                        post_j(j)

            # proj weights/bias DMA'd up front on the gpsimd DMA queue so
            # they don't delay the first pair's x/wq loads on SP
            bias_bc = small.tile([128, C], FP32, tag="bias", bufs=1)
            nc.gpsimd.dma_start(out=bias_bc, in_=_bc_ap(b_proj[:], 128))
            w_projT = small.tile([128, CT, C], FP32R, tag="wproj", bufs=1)
            nc.gpsimd.dma_start(
                out=w_projT, in_=wpr.rearrange("(ko p) o -> p ko o", p=128)
            )

            NO = 384

            def emit_proj(i2_list, psum_tags):
                for n_p, i2 in enumerate(i2_list):
                    a_sb = small.tile(
                        [128, CT, 256], FP32R, tag="a_sb", bufs=2
                    )
                    nc.sync.dma_start(
                        out=a_sb,
                        in_=aT_d[:, i2 * 256:(i2 + 1) * 256].rearrange(
                            "(ko p) n -> p ko n", p=128
                        ),
                    )
                    for ii in range(2):
                        i = 2 * i2 + ii
                        y_sb = small.tile([128, C], FP32, tag="y_sb", bufs=2)
                        for half in range(2):
                            tg = psum_tags[(n_p * 4 + ii * 2 + half)
                                           % len(psum_tags)]
                            if tg == "qkvps":
                                psf = psum_qkv.tile(
                                    [128, NQ], FP32, tag="qkvps",
                                    name="qkvps")
                            elif tg == "spsX":
                                psf = psum_s.tile(
                                    [128, NQ], FP32, tag="spsX",
                                    name="spsX", bufs=1)
                            else:
                                psf = psum_av.tile(
                                    [128, NQ], FP32, tag="av")
                            ps = psf[:, 0:NO]
                            for k in range(CT):
                                nc.tensor.matmul(
                                    ps,
                                    a_sb[:, k, ii * 128:(ii + 1) * 128],
                                    w_projT[:, k, half * NO:(half + 1) * NO],
                                    start=(k == 0),
                                    stop=(k == CT - 1),
                                )
                            nc.vector.tensor_add(
                                y_sb[:, half * NO:(half + 1) * NO], ps,
                                bias_bc[:, half * NO:(half + 1) * NO]
                            )
                        nc.sync.dma_start(
                            out=y[i * 128:(i + 1) * 128, :],
                            in_=y_sb,
                        )

            def last_pair_post_j(j):
                # overlap the first half of the projection with the last
                # pair's attention; attention psum tags still busy, so only
                # the (free) qkv tag is used here
                if j == 1:
                    emit_proj([0, 1, 2, 3], ["qkvps"])

            for hp in range(H // 2):
                tiles = emit_qkv_vaug(hp)
                emit_attention(
                    hp, *tiles,
                    post_j=last_pair_post_j if hp == H // 2 - 1 else None,
                )

            # ---------- rest of the projection; attention psums now free
            emit_proj([4, 5, 6, 7], ["qkvps", "spsX", "av", "av"])
        const_cm.__exit__(None, None, None)
    lp.__exit__(None, None, None)

    nc.finalize()
    return nc


def get_nc():
    global _CACHED_NC
    if _CACHED_NC is None:
        _CACHED_NC = build()
    return _CACHED_NC


LAST_RESULT = None


def kernel(x, w_qkv, w_proj, b_proj, **run_kwargs):
    x = np.ascontiguousarray(np.asarray(x, dtype=np.float32))
    w_qkv = np.ascontiguousarray(np.asarray(w_qkv, dtype=np.float32))
    w_proj = np.ascontiguousarray(np.asarray(w_proj, dtype=np.float32))
    b_proj = np.ascontiguousarray(np.asarray(b_proj, dtype=np.float32))
    assert x.shape == (B, N, C)

    nc = get_nc()
    w_qkvT = np.ascontiguousarray(w_qkv.T)
    w_projT = np.ascontiguousarray(w_proj.T)
    # aT row d=0 of each head carries a folded +1*(Z*recip(Z)) from the
    # denominator row riding in the hi+lo add; cancel it here.
    b_eff = b_proj - w_proj[:, [h * D for h in range(H)]].sum(axis=1)
    b_eff = np.ascontiguousarray(b_eff.astype(np.float32))
    in_maps = [
        {
            "xT": np.ascontiguousarray(x[i].T),
            "w_qkvT": w_qkvT,
            "w_projT": w_projT,
            "b_proj": b_eff,
        }
        for i in range(B)
    ]
    res = run_bass_kernel_spmd(nc, in_maps, list(range(B)), **run_kwargs)
    global LAST_RESULT
    LAST_RESULT = res
    out = np.stack([res.results[i]["y"] for i in range(B)], axis=0)
    return out


if __name__ == "__main__":
    rng = np.random.default_rng(0)
    x = rng.standard_normal((B, N, C), dtype=np.float32)
    w_qkv = (rng.standard_normal((F, C)) * 0.02).astype(np.float32)
    w_proj = (rng.standard_normal((C, C)) * 0.02).astype(np.float32)
    b_proj = (rng.standard_normal((C,)) * 0.02).astype(np.float32)
    out = kernel(x=x, w_qkv=w_qkv, w_proj=w_proj, b_proj=b_proj)
    print("out", out.shape, out.dtype, float(np.abs(out).max()))


# revision 27
# speedup vs baseline: 1.2290x; 1.0247x over previous
"""Multi-head attention Trainium2 Bass kernel (fp8-DoubleRow version).

Problem: B=8, N=2048, C=768, H=12 heads, D=64 head dim.
  qkv = x @ w_qkv.T          -> [B, N, 3C]
  per head: softmax(q k^T / sqrt(D)) @ v
  y = attn_out @ w_proj.T + b_proj

Sharding: data parallel over batch - one batch element per NeuronCore.

Mixed-precision strategy (cost model: matmul = out_free x pe_cycle x cpr,
fp8 DoubleRow cpr=0.5 vs fp32r 1.0):
  - QKV: fp32r (exact q/k/v; fp8 here costs too much accuracy).
  - S = k^T q: fp8 DoubleRow. lhsT = k8 duplicated via a stride-0 block dim,
    rhs blocks = (q_hi, q_lo) -> S = k8^T (q_hi + q_lo): q at ~14-bit
    precision, k at fp8, half the fp32r PE cost.
  - exp: 12/16 kv-tiles on ScalarE (activation Exp -> fp8 out), 4/16 via a
    Schraudolph fast-exp: DVE affine to int16 (bf16 exponent bits), gpsimd
    converts bf16 -> fp8e4m3.
  - AV: fp8 DoubleRow pairing kv-tiles. lhsT M-dim = [v_hi d0-63 | ones |
    v_lo d1-63]: v at ~14-bit, denominator row free (psum row 64).
  - normalize: tmp = copy(av[64:128]) (denominator lands in tmp[0]),
    recip + partition_broadcast, t = av[0:64] + tmp (row d0 absorbs +denom
    -> exactly +1 after the reciprocal multiply; corrected by subtracting
    sum_h w_proj[:, 64h] from b_proj on the host), aT = t * bc_r.
  - proj: fp32r, unchanged from the fp32r kernel.
"""

import numpy as np

import concourse.bass as bass
import concourse.mybir as mybir
import concourse.tile as tile
from concourse import bacc
from concourse.bass_utils import run_bass_kernel_spmd
from concourse.masks import make_identity

B, N, C, H = 8, 2048, 768, 12
D = C // H            # 64
F = 3 * C             # 2304
NT = N // 128         # 16 seq tiles
CT = C // 128         # 6 channel tiles
NQ = 512              # query-chunk width (1 psum bank of fp32)
NCH = N // NQ         # 4 chunks
SCALE = float(D) ** -0.5

FP32 = mybir.dt.float32
FP32R = mybir.dt.float32r
FP8 = mybir.dt.float8e4
I16 = mybir.dt.int16
BF16 = mybir.dt.bfloat16
EXP = mybir.ActivationFunctionType.Exp
DR = mybir.MatmulPerfMode.DoubleRow
ADD = mybir.AluOpType.add
SUB = mybir.AluOpType.subtract
MULT = mybir.AluOpType.mult

# Schraudolph constants: bf16 bits = 128*log2(exp(s_eff)) + 127*128,
# s_eff = S_psum * SCALE -> bits = S * (128*SCALE/ln2) + 16256.
SCHR_A = 128.0 * SCALE / float(np.log(2.0))
SCHR_B = 16256.0
N_SCHR_GROUPS = 2      # of 8 groups of 2 kv-tiles: last N on DVE+Pool

_CACHED_NC = None


def _dup2(ap):
    """Insert a stride-0 [0,2] block dim after the partition dim."""
    return bass.AP(
        tensor=ap.tensor,
        offset=ap.offset,
        ap=[list(ap.ap[0]), [0, 2]] + [list(d) for d in ap.ap[1:]],
    )


def _bc_ap(dram_ap, parts):
    """Partition-broadcast a 1-D DRAM AP to [parts, len] via stride-0."""
    return bass.AP(
        tensor=dram_ap.tensor,
        offset=dram_ap.offset,
        ap=[[0, parts]] + [list(p) for p in dram_ap.ap],
    )


def build():
    nc = bacc.Bacc()
    x = nc.dram_tensor("xT", [C, N], FP32, kind="ExternalInput")
    w_qkv = nc.dram_tensor("w_qkvT", [C, F], FP32, kind="ExternalInput")
    w_proj = nc.dram_tensor("w_projT", [2 * C, C], FP32, kind="ExternalInput")
    b_proj = nc.dram_tensor("b_proj", [C], FP32, kind="ExternalInput")
    y = nc.dram_tensor("y", [N, C], FP32, kind="ExternalOutput")
    C2 = 2 * C
    aT_d = nc.dram_tensor("aT_scratch", [C2, N], FP32R)

    xr = x[:, :].bitcast(FP32R)
    wqr = w_qkv[:, :].bitcast(FP32R)
    wpr = w_proj[:, :].bitcast(FP32R)

    lp = nc.allow_low_precision("fp8 attention with hi/lo compensation")
    lp.__enter__()
    with tile.TileContext(nc) as tc:
        const_cm = tc.tile_pool(name="const", bufs=1)
        const = const_cm.__enter__()
        ident_f = const.tile([128, 128], FP32)
        make_identity(nc, ident_f)
        ident = const.tile([128, 128], FP32R)
        nc.vector.tensor_copy(ident, ident_f)
        xr3 = xr.rearrange("(ko p) n -> p ko n", p=128)
        wqr3 = wqr.rearrange("(ko p) f -> p ko f", p=128)

        with tc.tile_pool(name="hpool", bufs=2) as hpool, \
             tc.tile_pool(name="spool", bufs=1) as spool, \
             tc.tile_pool(name="small", bufs=2) as small, \
             tc.tile_pool(name="psum_s", bufs=2, space="PSUM") as psum_s, \
             tc.tile_pool(name="psum_av", bufs=2, space="PSUM") as psum_av, \
             tc.tile_pool(name="psum_qkv", bufs=1, space="PSUM") as psum_qkv:

            class QkvPhase:
                """QKV chunks for one head pair, emitted j-at-a-time so the
                matmuls/copies interleave with the previous pair's
                attention on every engine stream."""

                def __init__(self, hp, rot=("qkvps",)):
                    self.hp = hp
                    self.rot = rot
                    self.wq = hpool.tile(
                        [128, CT, 3, 128], FP32R, tag="wq", name="wq", bufs=1,
                    )
                    for idx, m in enumerate((hp, CT + hp, 2 * CT + hp)):
                        nc.sync.dma_start(
                            out=self.wq[:, :, idx, :],
                            in_=wqr3[:, :, m * 128:(m + 1) * 128],
                        )
                    self.qTt = hpool.tile([128, 2, N], FP8, tag="qT")
                    self.kTt = hpool.tile([128, N], FP8, tag="kT")
                    self.vTt = hpool.tile([128, N], FP32R, tag="vT", bufs=1)

                def emit_j(self, j):
                    xc = hpool.tile([128, CT, NQ], FP32R, tag="xc", name="xc")
                    nc.sync.dma_start(
                        out=xc, in_=xr3[:, :, j * NQ:(j + 1) * NQ]
                    )
                    js = slice(j * NQ, (j + 1) * NQ)
                    for idx in range(3):
                        tg = self.rot[idx % len(self.rot)]
                        if tg == "qkvps":
                            ps = psum_qkv.tile(
                                [128, NQ], FP32, tag="qkvps", name="qkvps"
                            )
                        else:
                            ps = psum_s.tile(
                                [128, NQ], FP32, tag="spsX", name="spsX",
                                bufs=1,
                            )
                        for k in range(CT):
                            nc.tensor.matmul(
                                ps,
                                self.wq[:, k, idx, :],
                                xc[:, k, :],
                                start=(k == 0),
                                stop=(k == CT - 1),
                            )
                        if idx == 0:      # q -> hi + lo fp8
                            nc.vector.tensor_copy(self.qTt[:, 0, js], ps)
                            nc.vector.tensor_tensor(
                                self.qTt[:, 1, js], ps, self.qTt[:, 0, js],
                                SUB,
                            )
                        elif idx == 1:    # k -> fp8
                            nc.vector.tensor_copy(self.kTt[:, js], ps)
                        else:             # v -> fp32r (transposed later)
                            nc.vector.tensor_copy(self.vTt[:, js], ps)

                def finish(self):
                    # v transposes; vaug = [v_hi d0-63 | ones | v_lo d1-63]
                    vaugs = []
                    for a in range(2):
                        vaug = hpool.tile([128, NT, 128], FP8, tag=f"vaug{a}")
                        nc.vector.memset(vaug[:, :, D:D + 1], 1.0)
                        vaugs.append(vaug)
                    for t0 in range(0, NT, 8):
                        pts = [
                            psum_av.tile(
                                [128, 8, D], FP32R, tag="av", name=f"pt{a}"
                            )
                            for a in range(2)
                        ]
                        for g in range(8):
                            t = t0 + g
                            for a in range(2):
                                lo = a * D
                                nc.tensor.transpose(
                                    pts[a][:, g, :],
                                    self.vTt[lo:lo + D,
                                             t * 128:(t + 1) * 128],
                                    ident[lo:lo + D, lo:lo + D],
                                )
                        for a in range(2):
                            nc.vector.tensor_copy(
                                vaugs[a][:, t0:t0 + 8, 0:D], pts[a]
                            )
                            nc.vector.tensor_tensor(
                                vaugs[a][:, t0:t0 + 8, D + 1:128],
                                pts[a][:, :, 1:D],
                                vaugs[a][:, t0:t0 + 8, 1:D],
                                SUB,
                            )
                    return self.qTt, self.kTt, vaugs

            def emit_attention(hp, qTt, kTt, vaugs, qkv_cb=None, post_j=None):
                # ---- attention per q-chunk. aT tiles hold the full 128-row
                # [hi | denom*r | lo] block; proj contracts the doubled
                # channel dim with host-duplicated w_proj rows.
                aTts = [
                    hpool.tile([128, N], FP32, tag=f"aTt{a}", name=f"aTt{a}")
                    for a in range(2)
                ]
                for j in range(NCH):
                    js = slice(j * NQ, (j + 1) * NQ)
                    expSs = [
                        spool.tile(
                            [128, NT, NQ], FP8,
                            tag=f"expS{a}", name=f"expS{a}", bufs=2,
                        )
                        for a in range(2)
                    ]
                    def s_mm(out_ap, a, t):
                        lo = a * D
                        kap = kTt[lo:lo + D, t * 128:(t + 1) * 128]
                        nc.tensor.matmul(
                            out_ap,
                            _dup2(kap),
                            qTt[lo:lo + D, :, js],
                            start=True,
                            stop=True,
                            perf_mode=DR,
                        )

                    def emit_act_group(g):
                        # tiles (2g, 2g+1) -> ScalarE exp, own sps0/1 ring.
                        # last pair runs on sps0 alone so sps1 is free for
                        # the overlapped projection
                        t = 2 * g
                        sps_ab = [
                            psum_s.tile(
                                [128, 2, NQ], FP32, tag=f"sps{a}",
                                name=f"sps{a}", bufs=1,
                            )
                            for a in range(2)
                        ]
                        for u in range(2):
                            for a in range(2):
                                s_mm(sps_ab[a][:, u, :], a, t + u)
                        for a in range(2):
                            nc.scalar.activation(
                                out=expSs[a][:, t:t + 2, :],
                                in_=sps_ab[a][:, :, :],
                                func=EXP,
                                scale=SCALE,
                            )

                    def emit_schr_tile(t):
                        # single kv-tile -> DVE schraudolph + Pool convert,
                        # separate 1-bank psum ring (never gates ScalarE)
                        for a in range(2):
                            spsx = psum_s.tile(
                                [128, NQ], FP32, tag="spsX",
                                name="spsX", bufs=1,
                            )
                            s_mm(spsx, a, t)
                            i16 = small.tile(
                                [128, NQ], I16, tag=f"i16_{a}",
                                name=f"i16_{a}",
                            )
                            nc.vector.tensor_scalar(
                                i16, spsx, SCHR_A, SCHR_B, MULT, ADD,
                            )
                            nc.gpsimd.tensor_copy(
                                expSs[a][:, t:t + 1, :],
                                i16.bitcast(BF16),
                            )

                    # act-ring tiles 0-11, schr tiles 12-15, interleaved so
                    # DVE picks up schraudolph work early
                    emit_schr_tile(12)
                    emit_act_group(0)
                    emit_schr_tile(13)
                    emit_act_group(1)
                    emit_schr_tile(14)
                    emit_act_group(2)
                    emit_schr_tile(15)
                    for g in range(3, 6):
                        emit_act_group(g)
                    for a in range(2):
                        av = psum_av.tile([128, NQ], FP32, tag="av")
                        for u in range(NT // 2):
                            nc.tensor.matmul(
                                av,
                                vaugs[a][:, 2 * u:2 * u + 2, :],
                                expSs[a][:, 2 * u:2 * u + 2, :],
                                start=(u == 0),
                                stop=(u == NT // 2 - 1),
                                perf_mode=DR,
                            )
                        # normalize all 128 rows at once (partition count is
                        # free); row 64 becomes denom*recip = 1 and hits a
                        # zero row of the extended w_proj
                        recip = small.tile([1, NQ], FP32, tag="recip")
                        nc.vector.reciprocal(recip, av[D:D + 1, :])
                        bc128 = small.tile([128, NQ], FP32, tag="bc128")
                        nc.gpsimd.partition_broadcast(bc128, recip)
                        nc.vector.tensor_tensor(
                            aTts[a][:, js], av, bc128, MULT
                        )
                    # half-pair aT stores let the proj overlap the tail of
                    # the last pair's attention
                    if j % 2 == 1:
                        js2 = slice((j - 1) * NQ, (j + 1) * NQ)
                        for a in range(2):
                            h = 2 * hp + a
                            nc.sync.dma_start(
                                out=aT_d[h * 128:(h + 1) * 128, js2].bitcast(
                                    FP32
                                ),
                                in_=aTts[a][:, js2],
                            )
                    if qkv_cb is not None:
                        qkv_cb(j)
                    if post_j is not None:
                        post_j(j)

            # proj weights/bias DMA'd up front on the gpsimd DMA queue so
            # they don't delay the first pair's x/wq loads on SP
            bias_bc = small.tile([128, C], FP32, tag="bias", bufs=1)
            nc.gpsimd.dma_start(out=bias_bc, in_=_bc_ap(b_proj[:], 128))
            w_projT = small.tile([128, 2 * CT, C], FP32R, tag="wproj",
                                 bufs=1)
            nc.gpsimd.dma_start(
                out=w_projT, in_=wpr.rearrange("(ko p) o -> p ko o", p=128)
            )

            NO = 384

            def emit_proj(i2_list, psum_tags):
                for n_p, i2 in enumerate(i2_list):
                    a_sb = small.tile(
                        [128, 2 * CT, 256], FP32R, tag="a_sb", bufs=2
                    )
                    nc.sync.dma_start(
                        out=a_sb,
                        in_=aT_d[:, i2 * 256:(i2 + 1) * 256].rearrange(
                            "(ko p) n -> p ko n", p=128
                        ),
                    )
                    for ii in range(2):
                        i = 2 * i2 + ii
                        y_sb = small.tile([128, C], FP32, tag="y_sb", bufs=2)
                        for half in range(2):
                            tg = psum_tags[(n_p * 4 + ii * 2 + half)
                                           % len(psum_tags)]
                            if tg == "qkvps":
                                psf = psum_qkv.tile(
                                    [128, NQ], FP32, tag="qkvps",
                                    name="qkvps")
                            elif tg == "spsX":
                                psf = psum_s.tile(
                                    [128, NQ], FP32, tag="spsX",
                                    name="spsX", bufs=1)
                            elif tg in ("sps0", "sps1"):
                                psf = psum_s.tile(
                                    [128, 2, NQ], FP32, tag=tg,
                                    name=tg, bufs=1)[:, 0, :]
                            else:
                                psf = psum_av.tile(
                                    [128, NQ], FP32, tag="av")
                            ps = psf[:, 0:NO]
                            for k in range(2 * CT):
                                nc.tensor.matmul(
                                    ps,
                                    a_sb[:, k, ii * 128:(ii + 1) * 128],
                                    w_projT[:, k, half * NO:(half + 1) * NO],
                                    start=(k == 0),
                                    stop=(k == CT - 1),
                                )
                            nc.vector.tensor_add(
                                y_sb[:, half * NO:(half + 1) * NO], ps,
                                bias_bc[:, half * NO:(half + 1) * NO]
                            )
                        nc.sync.dma_start(
                            out=y[i * 128:(i + 1) * 128, :],
                            in_=y_sb,
                        )

            def last_pair_post_j(j):
                # overlap most of the projection with the last pair's
                # attention, using the qkv psum bank plus the freed sps1 ring
                pass

            cur = QkvPhase(0, rot=("qkvps", "spsX"))
            for j in range(NCH):
                cur.emit_j(j)
            tiles = cur.finish()
            for hp in range(H // 2):
                nxt = QkvPhase(hp + 1) if hp + 1 < H // 2 else None
                emit_attention(
                    hp, *tiles,
                    qkv_cb=nxt.emit_j if nxt is not None else None,
                    post_j=last_pair_post_j if hp == H // 2 - 1 else None,
                )
                if nxt is not None:
                    tiles = nxt.finish()

            # ---------- rest of the projection; attention psums now free
            emit_proj([0, 1, 2, 3, 4, 5, 6, 7],
                      ["qkvps", "sps0", "sps1", "spsX", "av"])
        const_cm.__exit__(None, None, None)
    lp.__exit__(None, None, None)

    nc.finalize()
    return nc


def get_nc():
    global _CACHED_NC
    if _CACHED_NC is None:
        _CACHED_NC = build()
    return _CACHED_NC


LAST_RESULT = None


def kernel(x, w_qkv, w_proj, b_proj, **run_kwargs):
    x = np.ascontiguousarray(np.asarray(x, dtype=np.float32))
    w_qkv = np.ascontiguousarray(np.asarray(w_qkv, dtype=np.float32))
    w_proj = np.ascontiguousarray(np.asarray(w_proj, dtype=np.float32))
    b_proj = np.ascontiguousarray(np.asarray(b_proj, dtype=np.float32))
    assert x.shape == (B, N, C)

    nc = get_nc()
    w_qkvT = np.ascontiguousarray(w_qkv.T)
    # extended proj weights: per head the aT block is
    # [hi d0-63 | denom*r (==1) | v_lo d1-63]; duplicate w rows for the lo
    # part, zero the denom row.
    w_ext = np.zeros((2 * C, C), dtype=np.float32)
    for h in range(H):
        blk = w_proj[:, h * D:(h + 1) * D]          # [C_out, 64]
        w_ext[h * 128:h * 128 + 64, :] = blk.T
        w_ext[h * 128 + 65:h * 128 + 128, :] = blk.T[1:64]
    w_projT = np.ascontiguousarray(w_ext)
    b_eff = np.ascontiguousarray(b_proj.astype(np.float32))
    in_maps = [
        {
            "xT": np.ascontiguousarray(x[i].T),
            "w_qkvT": w_qkvT,
            "w_projT": w_projT,
            "b_proj": b_eff,
        }
        for i in range(B)
    ]
    res = run_bass_kernel_spmd(nc, in_maps, list(range(B)), **run_kwargs)
    global LAST_RESULT
    LAST_RESULT = res
    out = np.stack([res.results[i]["y"] for i in range(B)], axis=0)
    return out


if __name__ == "__main__":
    rng = np.random.default_rng(0)
    x = rng.standard_normal((B, N, C), dtype=np.float32)
    w_qkv = (rng.standard_normal((F, C)) * 0.02).astype(np.float32)
    w_proj = (rng.standard_normal((C, C)) * 0.02).astype(np.float32)
    b_proj = (rng.standard_normal((C,)) * 0.02).astype(np.float32)
    out = kernel(x=x, w_qkv=w_qkv, w_proj=w_proj, b_proj=b_proj)
    print("out", out.shape, out.dtype, float(np.abs(out).max()))
